# revision 1
# baseline (speedup 1.0000x reference)
import sys as _sys
import os as _os

for _p in ("/opt/trn_rl_repo", _os.path.expanduser("~/.axon_site/_ro/trn_rl_repo")):
    if _os.path.isdir(_p) and _p not in _sys.path:
        _sys.path.append(_p)

"""Builder for the sliding-window attention kernel (NaiveHybridAttention).

Per-core program (SPMD, head-sharded):
  inputs (per core): xT (B,D,S), wqT/wkT/wvT (D,E), woT (E,D),
                     cos/sin RoPE tables (HD,S), additive masks (128,768)
  output: part (B,S,D) = this core's heads' contribution to the final
          out-projection; host sums the 8 partials.

Pipeline per batch:
  A) QKV: qT,kT = W^T-stationary matmuls -> [e, S] (RoPE fused into PSUM
     evacuation, scale folded into q tables); v = x-stationary -> [s, e].
  B) Attention per head: scores [q,k] in 256-query blocks over a <=768
     key span, additive window mask, exp (+row-sum via accum_out) on ACT,
     normalize, PE-transpose probs, av accumulates attnT [hd, q].
  C) Out-proj: attnT-stationary -> psum [s, o] -> DMA to part.

All matmuls run as float32r (full fp32 storage; 1 cycle/row at N>=256).
PSUM lives in one pool with 8 explicitly-tagged bank-sized slots shared
across phases (T1..T8).
"""

import os

import numpy as np
import concourse.bass as bass
from concourse import mybir

USE_GPSIMD_ADD = os.environ.get("NHA_GPSIMD_ADD", "0") == "1"
USE_TTR = os.environ.get("NHA_TTR", "0") == "1"
USE_F32R = os.environ.get("NHA_F32R", "1") == "1"

F32 = mybir.dt.float32
F32R = mybir.dt.float32r if USE_F32R else mybir.dt.float32
ROPE_BASE = 10000.0
WINDOW = 512
MASKW = 768
NEG = -10000.0


def r32(ap):
    return ap.bitcast(F32R) if USE_F32R else ap


def host_tables(S, HD=128):
    """cos/sin tables in transposed layout [HD, S]; sin is sign-folded so
    q_rope = q*cos_t + swap_halves(q)*sin_sg. Unscaled — the softmax 1/sqrt(HD)
    is applied via the Exp activation's scale parameter."""
    inv_freq = 1.0 / (ROPE_BASE ** (np.arange(0, HD, 2, dtype=np.float64) / HD))
    fr = np.arange(S, dtype=np.float64)[None, :] * inv_freq[:, None]  # [HD/2, S]
    cos = np.cos(fr)
    sin = np.sin(fr)
    cos_t = np.concatenate([cos, cos], 0).astype(np.float32)
    sin_sg = np.concatenate([-sin, sin], 0).astype(np.float32)
    return cos_t, sin_sg


def host_masks():
    """Multiplicative (1.0 valid / 0.0 invalid) sliding-window masks, applied
    to exp(scores) on the DVE (fused with the row-sum)."""
    r = np.arange(128)[:, None]
    c = np.arange(MASKW)[None, :]
    maskA = ((c >= r + 1) & (c <= r + 512)).astype(np.float32)
    maskB = ((c >= r + 129) & (c <= r + 640)).astype(np.float32)
    return maskA, maskB


def partial_ref_np(x, wq_r, wk_r, wv_r, wo_t):
    """NumPy mirror of the per-core computation (fp32).
    x: (B,S,D); wq_r/wk_r/wv_r: (E,D) row-slices of w_qkv; wo_t: (E,D) =
    w_out[:, e_slice].T. Returns (B,S,D) partial."""
    B, S, D = x.shape
    E = wq_r.shape[0]
    HC = E // 128
    q = np.einsum("bsd,ed->bse", x, wq_r).reshape(B, S, HC, 128)
    k = np.einsum("bsd,ed->bse", x, wk_r).reshape(B, S, HC, 128)
    v = np.einsum("bsd,ed->bse", x, wv_r).reshape(B, S, HC, 128)
    inv_freq = 1.0 / (ROPE_BASE ** (np.arange(0, 128, 2, dtype=np.float64) / 128))
    fr = np.arange(S, dtype=np.float64)[:, None] * inv_freq[None, :]
    emb = np.concatenate([fr, fr], -1)
    cos = np.cos(emb).astype(np.float32)[None, :, None, :]
    sin = np.sin(emb).astype(np.float32)[None, :, None, :]

    def rot(t):
        t1, t2 = t[..., :64], t[..., 64:]
        return np.concatenate([-t2, t1], -1)

    q = q * cos + rot(q) * sin
    k = k * cos + rot(k) * sin
    scale = 1.0 / np.sqrt(128.0)
    i = np.arange(S)[:, None]
    j = np.arange(S)[None, :]
    valid = (i - j >= 0) & (i - j < WINDOW)
    out = np.zeros((B, S, E), np.float32)
    for b in range(B):
        for h in range(HC):
            s = (q[b, :, h] @ k[b, :, h].T) * scale
            s = np.where(valid, s, -np.inf)
            s = s - s.max(-1, keepdims=True)
            p = np.exp(s)
            p /= p.sum(-1, keepdims=True)
            out[b, :, h * 128 : (h + 1) * 128] = p @ v[b, :, h]
    return np.einsum("bse,ed->bsd", out, wo_t).astype(np.float32)


def declare_io(nc, B, S, D, E):
    dt = F32
    t = {}
    t["xt"] = nc.dram_tensor("xt", [B, D, S], dt, kind="ExternalInput").ap()
    for n in ("wqt", "wkt", "wvt"):
        t[n] = nc.dram_tensor(n, [D, E], dt, kind="ExternalInput").ap()
    t["wot"] = nc.dram_tensor("wot", [E, D], dt, kind="ExternalInput").ap()
    for n in ("cost", "sint"):
        t[n] = nc.dram_tensor(n, [128, S], dt, kind="ExternalInput").ap()
    t["maskA"] = nc.dram_tensor("maskA", [128, MASKW], dt, kind="ExternalInput").ap()
    t["maskB"] = nc.dram_tensor("maskB", [128, MASKW], dt, kind="ExternalInput").ap()
    t["part"] = nc.dram_tensor("part", [B, S, D], dt, kind="ExternalOutput").ap()
    return t


def build_program(ctx, nc, tc, io, B, S, D, HC, reps=1):
    """Emit the per-core program. HC = heads on this core; E = HC*128.
    reps > 1 wraps the body in a hardware loop repeating the identical
    computation (for timing measurements); output is unchanged."""
    E = HC * 128
    KT = D // 128  # contraction tiles for qkv
    SC = S // 512  # s-chunks for qkv
    QB = S // 256  # query blocks for attention
    ST = S // 128
    OCW = min(512, D)
    OC = D // OCW

    const = ctx.enter_context(tc.tile_pool(name="const", bufs=1))
    work = ctx.enter_context(tc.tile_pool(name="work", bufs=1))
    xsp = ctx.enter_context(tc.tile_pool(name="xs", bufs=5))
    tmp = ctx.enter_context(tc.tile_pool(name="tmp", bufs=2))
    smp = ctx.enter_context(tc.tile_pool(name="sm", bufs=1))
    pp = ctx.enter_context(tc.tile_pool(name="pp", bufs=2))
    rp = ctx.enter_context(tc.tile_pool(name="rp", bufs=4))
    ptp = ctx.enter_context(tc.tile_pool(name="pt", bufs=1))
    outp = ctx.enter_context(tc.tile_pool(name="outp", bufs=2))
    ps = ctx.enter_context(tc.tile_pool(name="ps", bufs=1, space="PSUM"))

    # ---- constants ----
    # q/k/v weights: one DMA per 128-row k-tile so the first matmuls only
    # depend on the slices they read (kills the startup stall). Other consts
    # go on the gpsimd (SWDGE) queue to stay off the HWDGE queue that
    # streams x.
    wq_sb = const.tile([128, KT, E], F32R)
    wk_sb = const.tile([128, KT, E], F32R)
    wv_sb = const.tile([128, KT, E], F32R)
    for kt in range(KT):
        rows = bass.ts(kt, 128)
        nc.gpsimd.dma_start(wq_sb[:, kt, :], r32(io["wqt"][rows, :]))
        nc.gpsimd.dma_start(wk_sb[:, kt, :], r32(io["wkt"][rows, :]))
        nc.gpsimd.dma_start(wv_sb[:, kt, :], r32(io["wvt"][rows, :]))
    wo_sb = const.tile([128, HC, D], F32R)
    nc.gpsimd.dma_start(wo_sb[:], r32(io["wot"].rearrange("(et p) o -> p et o", p=128)))
    cost = const.tile([128, S], F32)
    nc.gpsimd.dma_start(cost[:], io["cost"][:])
    sint = const.tile([128, S], F32)
    nc.gpsimd.dma_start(sint[:], io["sint"][:])
    mA = const.tile([128, MASKW], F32)
    nc.gpsimd.dma_start(mA[:], io["maskA"][:])
    mB = const.tile([128, MASKW], F32)
    nc.gpsimd.dma_start(mB[:], io["maskB"][:])
    ident = const.tile([128, 128], F32)
    from concourse.masks import make_identity

    make_identity(nc, ident[:])

    def rope(dst, src_ps, cos_t, sin_t, cols, w):
        """dst[:, cols] = src_ps*cos + swap_halves(src_ps)*sin (RoPE).
        Muls (PSUM readers) on DVE; final SBUF-only add on GpSimd to keep
        the DVE burst at chunk boundaries short."""
        rot = tmp.tile([128, 512], F32, tag="rot")
        nc.vector.tensor_mul(rot[0:64, :w], src_ps[64:128, :w], sin_t[0:64, cols])
        nc.vector.tensor_mul(rot[64:128, :w], src_ps[0:64, :w], sin_t[64:128, cols])
        cv = tmp.tile([128, 512], F32, tag="cosv")
        nc.vector.tensor_mul(cv[:, :w], src_ps[:, :w], cos_t[:, cols])
        if USE_GPSIMD_ADD:
            nc.gpsimd.tensor_add(dst, cv[:, :w], rot[:, :w])
        else:
            nc.vector.tensor_add(dst, cv[:, :w], rot[:, :w])

    def body():
        _emit_body(nc, tc, io, B, S, D, HC, locals_=dict(
            const=const, work=work, xsp=xsp, tmp=tmp, smp=smp, pp=pp, rp=rp,
            ptp=ptp, outp=outp, ps=ps,
            wq_sb=wq_sb, wk_sb=wk_sb, wv_sb=wv_sb, wo_sb=wo_sb,
            cost=cost, sint=sint, mA=mA, mB=mB, ident=ident, rope=rope,
        ))

    if reps > 1:
        with tc.For_i(0, reps, 1):
            body()
    else:
        body()


def _emit_body(nc, tc, io, B, S, D, HC, locals_):
    E = HC * 128
    KT = D // 128
    SC = S // 512
    QB = S // 256
    ST = S // 128
    OCW = min(512, D)
    OC = D // OCW
    const = locals_["const"]; work = locals_["work"]; xsp = locals_["xsp"]
    tmp = locals_["tmp"]; smp = locals_["smp"]; pp = locals_["pp"]
    rp = locals_["rp"]; ptp = locals_["ptp"]; outp = locals_["outp"]
    ps = locals_["ps"]
    wq_sb = locals_["wq_sb"]; wk_sb = locals_["wk_sb"]; wv_sb = locals_["wv_sb"]
    wo_sb = locals_["wo_sb"]; cost = locals_["cost"]; sint = locals_["sint"]
    mA = locals_["mA"]; mB = locals_["mB"]; ident = locals_["ident"]
    rope = locals_["rope"]

    for b in range(B):
        # ---- A) QKV projection ----
        qT = work.tile([128, HC, S], F32R, tag="qT")
        kT = work.tile([128, HC, S], F32R, tag="kT")
        v_sb = work.tile([128, ST, E], F32R, tag="v")
        for sc in range(SC):
            cols = bass.ts(sc, 512)
            q_ps = [
                ps.tile([128, 512], F32, tag=t, name=f"q_ps{i}")
                for i, t in enumerate(("T1", "T2")[:HC])
            ]
            k_ps = [
                ps.tile([128, 512], F32, tag=t, name=f"k_ps{i}")
                for i, t in enumerate(("T3", "T4")[:HC])
            ]
            v_ps = [
                ps.tile([128, E], F32, tag=t, name=f"v_ps{i}")
                for i, t in enumerate(("T5", "T6", "T7", "T8"))
            ]
            for kt in range(KT):
                xs = xsp.tile([128, 512], F32R)
                nc.sync.dma_start(xs[:], r32(io["xt"][b, bass.ts(kt, 128), cols]))
                f = dict(start=(kt == 0), stop=(kt == KT - 1))
                # v first: its psum slots are evacuated fastest, so the next
                # chunk's accumulation can begin while q/k RoPE evac runs
                for ss in range(4):
                    nc.tensor.matmul(
                        v_ps[ss][:],
                        xs[:, bass.ts(ss, 128)],
                        wv_sb[:, kt, :],
                        **f,
                    )
                for et in range(HC):
                    nc.tensor.matmul(
                        k_ps[et][:], wk_sb[:, kt, bass.ts(et, 128)], xs[:], **f
                    )
                    nc.tensor.matmul(
                        q_ps[et][:], wq_sb[:, kt, bass.ts(et, 128)], xs[:], **f
                    )
            for et in range(HC):
                rope(qT[:, et, cols], q_ps[et], cost, sint, cols, 512)
                rope(kT[:, et, cols], k_ps[et], cost, sint, cols, 512)
            for ss in range(4):
                nc.scalar.copy(v_sb[:, sc * 4 + ss, :], v_ps[ss][:])

        # ---- B) attention, per head ----
        # Pipelined per 256-query block: both subtiles' score matmuls are
        # issued before either softmax, so PE stays busy during the
        # mask+exp+normalize chain (in-order PE stream). Score psums use 4
        # tags (A/B piece x 2 subtiles); in-place mask-add in PSUM; exp
        # (+row-sum) reads PSUM directly on ACT.
        attnT = work.tile([128, HC, S], F32R, tag="attnT")
        exp_scale = float(1.0 / np.sqrt(128.0))

        def emit_qk(h, qb):
            """Score matmuls for both 128-query subtiles of block qb."""
            q0 = qb * 256
            kstart = max(0, q0 - WINDOW)
            kspan = q0 + 256 - kstart
            la = min(512, kspan)
            lb = kspan - la
            pieces = {}
            for sub in range(2):
                qcols = bass.ds(q0 + sub * 128, 128)
                spA = ps.tile(
                    [128, 512], F32, tag=("T1", "T2")[sub], name=f"spA{sub}"
                )
                nc.tensor.matmul(
                    spA[:, :la],
                    qT[:, h, qcols],
                    kT[:, h, bass.ds(kstart, la)],
                )
                pieces[sub] = [(spA, 0, la)]
                if lb:
                    spB = ps.tile(
                        [128, 256], F32, tag=("T5", "T6")[sub], name=f"spB{sub}"
                    )
                    nc.tensor.matmul(
                        spB[:, :lb],
                        qT[:, h, qcols],
                        kT[:, h, bass.ds(kstart + 512, lb)],
                    )
                    pieces[sub].append((spB, la, lb))
            return dict(h=h, q0=q0, kstart=kstart, kspan=kspan, pieces=pieces)

        def emit_rest(d):
            """Softmax + transposes + AV for a previously-issued block."""
            h, q0, kstart, kspan = d["h"], d["q0"], d["kstart"], d["kspan"]
            nkt = kspan // 128
            pTs = [
                ptp.tile([128, 256], F32R, tag=f"pT{i}", name=f"pT{i}")
                for i in range(nkt)
            ]
            p_all = {}
            for sub in range(2):
                msk = mA if sub == 0 else mB
                rsums = []
                p_sbs = []
                for pi, (sp, off, ln) in enumerate(d["pieces"][sub]):
                    # exp straight from PSUM (releases the score slot ASAP);
                    # window mask applied multiplicatively, fused with the
                    # row-sum, in one DVE op
                    p_sb = pp.tile(
                        [128, 512 if pi == 0 else 256],
                        F32,
                        tag=f"p{sub}{pi}",
                        name=f"p{sub}{pi}",
                    )
                    nc.scalar.activation(
                        p_sb[:, :ln],
                        sp[:, :ln],
                        mybir.ActivationFunctionType.Exp,
                        scale=exp_scale,
                    )
                    rs = rp.tile([128, 1], F32, tag=f"rs{sub}{pi}")
                    if USE_TTR:
                        nc.vector.tensor_tensor_reduce(
                            p_sb[:, :ln],
                            p_sb[:, :ln],
                            msk[:, bass.ds(MASKW - kspan + off, ln)],
                            1.0,
                            0.0,
                            mybir.AluOpType.mult,
                            mybir.AluOpType.add,
                            rs[:],
                        )
                    else:
                        nc.vector.tensor_mul(
                            p_sb[:, :ln],
                            p_sb[:, :ln],
                            msk[:, bass.ds(MASKW - kspan + off, ln)],
                        )
                        nc.vector.reduce_sum(
                            out=rs[:], in_=p_sb[:, :ln], axis=mybir.AxisListType.X
                        )
                    rsums.append(rs)
                    p_sbs.append((p_sb, off, ln))
                if len(rsums) == 2:
                    nc.vector.tensor_add(rsums[0][:], rsums[0][:], rsums[1][:])
                rinv = rp.tile([128, 1], F32, tag=f"rinv{sub}")
                nc.vector.reciprocal(rinv[:], rsums[0][:])
                for p_sb, off, ln in p_sbs:
                    nc.vector.tensor_scalar_mul(p_sb[:, :ln], p_sb[:, :ln], rinv[:])
                p_all[sub] = p_sbs
            for sub in range(2):
                for p_sb, off, ln in p_all[sub]:
                    for kk in range(ln // 128):
                        kt2 = (off + kk * 128) // 128
                        tp = ps.tile(
                            [128, 128],
                            F32,
                            tag=("T3" if kt2 % 2 == 0 else "T4"),
                            name=f"tp{kt2}",
                        )
                        nc.tensor.transpose(
                            tp[:],
                            p_sb[:, bass.ds(kk * 128, 128)],
                            ident[:],
                        )
                        nc.any.tensor_copy(pTs[kt2][:, bass.ts(sub, 128)], tp[:])
            av = ps.tile([128, 256], F32, tag="T7", name="av")
            for kt2 in range(nkt):
                nc.tensor.matmul(
                    av[:],
                    v_sb[:, kstart // 128 + kt2, bass.ts(h, 128)],
                    pTs[kt2][:],
                    start=(kt2 == 0),
                    stop=(kt2 == nkt - 1),
                )
            nc.any.tensor_copy(attnT[:, h, bass.ds(q0, 256)], av[:])

        # one-stage software pipeline: qk(i) is issued before the softmax/
        # transpose/AV of block i-1, so PE has work during the softmax chain
        prev = None
        for h in range(HC):
            for qb in range(QB):
                d = emit_qk(h, qb)
                if prev is not None:
                    emit_rest(prev)
                prev = d
        emit_rest(prev)

        # ---- C) out-projection (partial over this core's E dims) ----
        # evac copies alternate DVE/ACT; one batched 1 MB output DMA per
        # 128-row stripe instead of four 256 KB ones
        for st in range(ST):
            osb = outp.tile([128, D], F32, tag="osb")
            for oc in range(OC):
                o_ps = ps.tile(
                    [128, OCW], F32, tag=("T1" if oc % 2 == 0 else "T2"), name="o_ps"
                )
                for et in range(HC):
                    nc.tensor.matmul(
                        o_ps[:],
                        attnT[:, et, bass.ts(st, 128)],
                        wo_sb[:, et, bass.ts(oc, OCW)],
                        start=(et == 0),
                        stop=(et == HC - 1),
                    )
                dst = osb[:, bass.ts(oc, OCW)]
                if oc % 2 == 0:
                    nc.vector.tensor_copy(dst, o_ps[:])
                else:
                    nc.scalar.copy(dst, o_ps[:])
            nc.sync.dma_start(io["part"][b, bass.ts(st, 128), :], osb[:])


# ======================================================================
# 8-core SPMD wrapper
# ======================================================================
from contextlib import ExitStack as _ExitStack

N_CORES = 8
B_FULL, S_FULL, D_FULL, H_FULL, HD_FULL = 2, 2048, 2048, 16, 128
HC_FULL = H_FULL // N_CORES  # 2 heads per core

_nc_cache = {}


def get_compiled(reps=1):
    """Build + bacc-compile the per-core Bass program (cached per reps)."""
    if reps not in _nc_cache:
        import concourse.bacc as bacc
        from concourse import tile

        nc = bacc.Bacc(
            "TRN2", target_bir_lowering=False, debug=False, num_devices=N_CORES
        )
        io = declare_io(nc, B_FULL, S_FULL, D_FULL, HC_FULL * 128)
        with tile.TileContext(nc) as tc:
            with _ExitStack() as ctx:
                build_program(
                    ctx, nc, tc, io, B_FULL, S_FULL, D_FULL, HC_FULL, reps=reps
                )
        nc.compile()
        _nc_cache[reps] = nc
    return _nc_cache[reps]


def make_in_maps(x, w_qkv, w_out):
    """Host-side sharding: per-core input dicts (head-sharded)."""
    x = np.ascontiguousarray(np.asarray(x, dtype=np.float32))
    w_qkv = np.ascontiguousarray(np.asarray(w_qkv, dtype=np.float32))
    w_out = np.ascontiguousarray(np.asarray(w_out, dtype=np.float32))
    D = D_FULL
    xt = np.ascontiguousarray(x.transpose(0, 2, 1))
    cos_t, sin_t = host_tables(S_FULL)
    maskA, maskB = host_masks()
    in_maps = []
    for c in range(N_CORES):
        e0, e1 = c * HC_FULL * 128, (c + 1) * HC_FULL * 128
        in_maps.append(
            dict(
                xt=xt,
                wqt=np.ascontiguousarray(w_qkv[e0:e1].T),
                wkt=np.ascontiguousarray(w_qkv[D + e0 : D + e1].T),
                wvt=np.ascontiguousarray(w_qkv[2 * D + e0 : 2 * D + e1].T),
                wot=np.ascontiguousarray(w_out[:, e0:e1].T),
                cost=cos_t,
                sint=sin_t,
                maskA=maskA,
                maskB=maskB,
            )
        )
    return in_maps


def combine(parts):
    """Sum the 8 per-core out-projection partials."""
    acc = np.zeros((B_FULL, S_FULL, D_FULL), np.float64)
    for p in parts:
        acc += p
    return acc.astype(np.float32)


def kernel(x, w_qkv, w_out):
    from concourse import bass_utils

    nc = get_compiled(reps=1)
    in_maps = make_in_maps(x, w_qkv, w_out)
    res = bass_utils.run_bass_kernel_spmd(
        nc, in_maps, core_ids=list(range(N_CORES))
    )
    return combine([res.results[c]["part"] for c in range(N_CORES)])



# revision 7
# speedup vs baseline: 3.0919x; 3.0919x over previous
import sys as _sys
import os as _os

for _p in ("/opt/trn_rl_repo", _os.path.expanduser("~/.axon_site/_ro/trn_rl_repo")):
    if _os.path.isdir(_p) and _p not in _sys.path:
        _sys.path.append(_p)

"""Sliding-window attention kernel (NaiveHybridAttention), head-sharded SPMD.

Per-core program (2 of 16 heads per core):
  inputs: xT (B,D,S), wqT/wkT/wvT (D,E), woT (E,D), cos/sin RoPE tables
          (HD,S), window masks (128,512)x2
  output: part (B,S,D) = this core's heads' contribution to the final
          out-projection; host sums the 8 partials.

Pipeline per batch:
  A) QKV in 256-column chunks with parity-alternating PSUM tags: qT,kT via
     W-stationary matmuls (RoPE fused into PSUM evacuation on DVE),
     v via x-stationary matmuls -> v_sb [s,e].
  B) Attention per (head, 256-query block), scores TRANSPOSED [k,q]:
     per 128-key tile, matmul -> PSUM [k,256q]; Exp on ACT evacuates
     PSUM->SBUF; window mask as a single [128,512] multiplicative DVE op
     per masked bank (only 2 of 6 tiles per block need masking); AV
     accumulates stationary-v matmuls directly on the masked
     probabilities (queries stay in the free dim -> NO transposes);
     row sums via accumulating ones-matmuls -> [1,256]; reciprocal on
     DVE; partition_broadcast on Pool; one DVE multiply normalizes while
     evacuating av -> attnT.
  C) Out-projection: attnT-stationary matmuls -> psum [s,o] -> SBUF ->
     part via SWDGE (gpsimd queue, keeps the SP/HWDGE queue free for x).

All matmuls float32r (full fp32 storage, 1 cycle/row at free-dim>=256).
PSUM: 8 bank-sized slots, tags PS0..PS7 reused across phases.
"""

import os

import numpy as np
import concourse.bass as bass
from concourse import mybir

F32 = mybir.dt.float32
F32R = mybir.dt.float32r
ROPE_BASE = 10000.0
WINDOW = 512
NEG = -10000.0


def r32(ap):
    return ap.bitcast(F32R)


def host_tables(S, HD=128):
    """cos/sin tables in transposed layout [HD, S]; sin is sign-folded so
    q_rope = q*cos_t + swap_halves(q)*sin_sg."""
    inv_freq = 1.0 / (ROPE_BASE ** (np.arange(0, HD, 2, dtype=np.float64) / HD))
    fr = np.arange(S, dtype=np.float64)[None, :] * inv_freq[:, None]  # [HD/2, S]
    cos = np.cos(fr)
    sin = np.sin(fr)
    cos_t = np.concatenate([cos, cos], 0).astype(np.float32)
    sin_sg = np.concatenate([-sin, sin], 0).astype(np.float32)
    return cos_t, sin_sg


def host_masks():
    """Multiplicative masks in [key, query] layout, [128, 2, 256] flattened
    to [128, 512]. For a 256-query block starting at q0 with key tile at
    offset r = q0 - ktile_start: valid iff 0 <= r + qi - ki < 512.
      maskS = [r=512 | r=384] (start-of-window tiles)
      maskE = [r=0   | r=-128] (end-of-window / diagonal tiles)
    Interior tiles (r=128, 256) are fully valid and skip masking."""
    ki = np.arange(128)[:, None]
    qi = np.arange(256)[None, :]
    m512 = (qi < ki).astype(np.float32)
    m384 = (qi - ki < 128).astype(np.float32)
    m0 = (qi >= ki).astype(np.float32)
    mm128 = (qi >= ki + 128).astype(np.float32)
    maskS = np.concatenate([m512, m384], axis=1)  # [128, 512]
    maskE = np.concatenate([m0, mm128], axis=1)  # [128, 512]
    return maskS, maskE


def partial_ref_np(x, wq_r, wk_r, wv_r, wo_t):
    """NumPy mirror of the per-core computation (fp32).
    x: (B,S,D); wq_r/wk_r/wv_r: (E,D) row-slices of w_qkv; wo_t: (E,D) =
    w_out[:, e_slice].T. Returns (B,S,D) partial."""
    B, S, D = x.shape
    E = wq_r.shape[0]
    HC = E // 128
    q = np.einsum("bsd,ed->bse", x, wq_r).reshape(B, S, HC, 128)
    k = np.einsum("bsd,ed->bse", x, wk_r).reshape(B, S, HC, 128)
    v = np.einsum("bsd,ed->bse", x, wv_r).reshape(B, S, HC, 128)
    inv_freq = 1.0 / (ROPE_BASE ** (np.arange(0, 128, 2, dtype=np.float64) / 128))
    fr = np.arange(S, dtype=np.float64)[:, None] * inv_freq[None, :]
    emb = np.concatenate([fr, fr], -1)
    cos = np.cos(emb).astype(np.float32)[None, :, None, :]
    sin = np.sin(emb).astype(np.float32)[None, :, None, :]

    def rot(t):
        t1, t2 = t[..., :64], t[..., 64:]
        return np.concatenate([-t2, t1], -1)

    q = q * cos + rot(q) * sin
    k = k * cos + rot(k) * sin
    scale = 1.0 / np.sqrt(128.0)
    i = np.arange(S)[:, None]
    j = np.arange(S)[None, :]
    valid = (i - j >= 0) & (i - j < WINDOW)
    out = np.zeros((B, S, E), np.float32)
    for b in range(B):
        for h in range(HC):
            s = (q[b, :, h] @ k[b, :, h].T) * scale
            s = np.where(valid, s, -np.inf)
            s = s - s.max(-1, keepdims=True)
            p = np.exp(s)
            p /= p.sum(-1, keepdims=True)
            out[b, :, h * 128 : (h + 1) * 128] = p @ v[b, :, h]
    return np.einsum("bse,ed->bsd", out, wo_t).astype(np.float32)


def declare_io(nc, B, S, D, E):
    dt = F32
    t = {}
    t["xt"] = nc.dram_tensor("xt", [B, D, S], dt, kind="ExternalInput").ap()
    for n in ("wqt", "wkt", "wvt"):
        t[n] = nc.dram_tensor(n, [D, E], dt, kind="ExternalInput").ap()
    t["wot"] = nc.dram_tensor("wot", [E, D], dt, kind="ExternalInput").ap()
    for n in ("cost", "sint"):
        t[n] = nc.dram_tensor(n, [128, S], dt, kind="ExternalInput").ap()
    t["maskS"] = nc.dram_tensor("maskS", [128, 512], dt, kind="ExternalInput").ap()
    t["maskE"] = nc.dram_tensor("maskE", [128, 512], dt, kind="ExternalInput").ap()
    t["part"] = nc.dram_tensor("part", [B, S, D], dt, kind="ExternalOutput").ap()
    return t


def build_program(ctx, nc, tc, io, B, S, D, HC, reps=1):
    """Emit the per-core program. HC = heads on this core; E = HC*128."""
    E = HC * 128
    KT = D // 128  # contraction tiles for qkv

    const = ctx.enter_context(tc.tile_pool(name="const", bufs=1))
    work = ctx.enter_context(tc.tile_pool(name="work", bufs=1))
    xsp = ctx.enter_context(tc.tile_pool(name="xs", bufs=3))
    tmp = ctx.enter_context(tc.tile_pool(name="tmp", bufs=2))
    pp = ctx.enter_context(tc.tile_pool(name="pp", bufs=1))
    rp = ctx.enter_context(tc.tile_pool(name="rp", bufs=1))
    outp = ctx.enter_context(tc.tile_pool(name="outp", bufs=2))
    ps = ctx.enter_context(tc.tile_pool(name="ps", bufs=1, space="PSUM"))

    # ---- constants ----
    # qkv weights one DMA per 128-row k-tile (first matmuls depend only on
    # the slices they read); all consts on the gpsimd (SWDGE) queue to stay
    # off the HWDGE queue that streams x.
    wq_sb = const.tile([128, KT, E], F32R)
    wk_sb = const.tile([128, KT, E], F32R)
    wv_sb = const.tile([128, KT, E], F32R)
    for kt in range(KT):
        rows = bass.ts(kt, 128)
        nc.gpsimd.dma_start(wq_sb[:, kt, :], r32(io["wqt"][rows, :]))
        nc.gpsimd.dma_start(wk_sb[:, kt, :], r32(io["wkt"][rows, :]))
        nc.gpsimd.dma_start(wv_sb[:, kt, :], r32(io["wvt"][rows, :]))
    wo_sb = const.tile([128, HC, D], F32R)
    nc.gpsimd.dma_start(wo_sb[:], r32(io["wot"].rearrange("(et p) o -> p et o", p=128)))
    cost = const.tile([128, S], F32)
    nc.gpsimd.dma_start(cost[:], io["cost"][:])
    sint = const.tile([128, S], F32)
    nc.gpsimd.dma_start(sint[:], io["sint"][:])
    mS = const.tile([128, 512], F32)
    nc.gpsimd.dma_start(mS[:], io["maskS"][:])
    mE = const.tile([128, 512], F32)
    nc.gpsimd.dma_start(mE[:], io["maskE"][:])
    ones = const.tile([128, 1], F32)
    nc.vector.memset(ones[:], 1.0)

    def rope(dst, src_ps, cols):
        """dst[:, cols] = src_ps*cos + swap_halves(src_ps)*sin (RoPE), 256
        cols. Muls (PSUM readers) on DVE; final add on DVE too."""
        rot = tmp.tile([128, 256], F32, tag="rot")
        nc.vector.tensor_mul(rot[0:64, :], src_ps[64:128, :], sint[0:64, cols])
        nc.vector.tensor_mul(rot[64:128, :], src_ps[0:64, :], sint[64:128, cols])
        cv = tmp.tile([128, 256], F32, tag="cosv")
        nc.vector.tensor_mul(cv[:, :], src_ps[:, :], cost[:, cols])
        nc.vector.tensor_add(dst, cv[:, :], rot[:, :])

    def body():
        _emit_body(nc, tc, io, B, S, D, HC, locals_=dict(
            const=const, work=work, xsp=xsp, tmp=tmp, pp=pp, rp=rp,
            outp=outp, ps=ps,
            wq_sb=wq_sb, wk_sb=wk_sb, wv_sb=wv_sb, wo_sb=wo_sb,
            cost=cost, sint=sint, mS=mS, mE=mE, ones=ones, rope=rope,
        ))

    if reps > 1:
        with tc.For_i(0, reps, 1):
            body()
    else:
        body()


def _emit_body(nc, tc, io, B, S, D, HC, locals_):
    E = HC * 128
    KT = D // 128
    CW = 256  # qkv chunk width
    SC = S // CW  # chunks per batch
    QB = S // 256  # 256-query attention blocks per head
    ST = S // 128
    OCW = 512
    OC = D // OCW
    const = locals_["const"]; work = locals_["work"]; xsp = locals_["xsp"]
    tmp = locals_["tmp"]; pp = locals_["pp"]; rp = locals_["rp"]
    outp = locals_["outp"]; ps = locals_["ps"]
    wq_sb = locals_["wq_sb"]; wk_sb = locals_["wk_sb"]; wv_sb = locals_["wv_sb"]
    wo_sb = locals_["wo_sb"]; cost = locals_["cost"]; sint = locals_["sint"]
    mS = locals_["mS"]; mE = locals_["mE"]; ones = locals_["ones"]
    rope = locals_["rope"]

    exp_scale = float(1.0 / np.sqrt(128.0))

    for b in range(B):
        # ---- A) QKV projection, 256-col chunks, parity psum tags ----
        qT = work.tile([128, HC, S], F32R, tag="qT")
        kT = work.tile([128, HC, S], F32R, tag="kT")
        v_sb = work.tile([128, ST, E], F32R, tag="v")
        for sc in range(SC):
            cols = bass.ds(sc * CW, CW)
            par = sc % 2
            q_ps = ps.tile([128, HC, 256], F32, tag=f"PS{par}", name=f"q_ps{par}")
            k_ps = ps.tile([128, HC, 256], F32, tag=f"PS{2+par}", name=f"k_ps{par}")
            v_ps = ps.tile([128, 2, E], F32, tag=f"PS{4+par}", name=f"v_ps{par}")
            for kt4 in range(KT // 4):
                # one DMA covers 4 contraction k-tiles: [512 rows, 256 cols]
                xs = xsp.tile([128, 4, 256], F32R)
                nc.sync.dma_start(
                    xs[:],
                    r32(
                        io["xt"][b, bass.ts(kt4, 512), cols].rearrange(
                            "(four p) c -> p four c", p=128
                        )
                    ),
                )
                for kk in range(4):
                    kt = kt4 * 4 + kk
                    # Two logical accumulations share each psum bank as ONE
                    # group: start only on the bank's first matmul (clears
                    # has_written for the whole 2KB zero region), stop only
                    # on its last.
                    # v first: its psum is evacuated fastest
                    for ss in range(2):
                        nc.tensor.matmul(
                            v_ps[:, ss, :],
                            xs[:, kk, bass.ts(ss, 128)],
                            wv_sb[:, kt, :],
                            start=(kt == 0 and ss == 0),
                            stop=(kt == KT - 1 and ss == 1),
                        )
                    for et in range(HC):
                        nc.tensor.matmul(
                            k_ps[:, et, :],
                            wk_sb[:, kt, bass.ts(et, 128)],
                            xs[:, kk, :],
                            start=(kt == 0 and et == 0),
                            stop=(kt == KT - 1 and et == HC - 1),
                        )
                        nc.tensor.matmul(
                            q_ps[:, et, :],
                            wq_sb[:, kt, bass.ts(et, 128)],
                            xs[:, kk, :],
                            start=(kt == 0 and et == 0),
                            stop=(kt == KT - 1 and et == HC - 1),
                        )
            for et in range(HC):
                rope(qT[:, et, cols], q_ps[:, et, :], cols)
                rope(kT[:, et, cols], k_ps[:, et, :], cols)
            nc.scalar.copy(v_sb[:, sc * 2 : sc * 2 + 2, :], v_ps[:])

        # ---- B) attention, transposed scores [k, q] ----
        attnT = work.tile([128, HC, S], F32, tag="attnT")

        def emit_scores(h, qb):
            """Score matmuls + exp + masks for block qb (256 queries).
            Returns state for emit_av. Scores land in PSUM [128k, 256q]
            tiles, two k-tiles per [128,512] bank; Exp evacuates to the
            SBUF p tile; the 2 boundary banks get one [128,512] mask-mul
            each on DVE. Banks are emitted in av-consumption order:
            interior (unmasked) first."""
            q0 = qb * 256
            kstart = max(0, q0 - WINDOW)
            nkt = (q0 + 256 - kstart) // 128
            nbk = nkt // 2
            par = qb % 2
            mov = qT[:, h, bass.ds(q0, 256)]
            p_sb = pp.tile([128, 3, 512], F32, tag=f"p{par}", name=f"p{par}")
            # bank order: interior first (no mask -> av can start earliest)
            if nbk == 3:
                order = [1, 0, 2]
                bmask = {0: mS, 2: mE}
            elif nbk == 2:
                order = [0, 1]
                bmask = {1: mE}
            else:
                order = [0]
                bmask = {0: mE}
            for bk in order:
                sp = ps.tile(
                    [128, 512], F32, tag=f"PS{3*par + order.index(bk)}",
                    name=f"sp{bk}",
                )
                for half in range(2):
                    kt = bk * 2 + half
                    nc.tensor.matmul(
                        sp[:, bass.ts(half, 256)],
                        kT[:, h, bass.ds(kstart + kt * 128, 128)],
                        mov,
                    )
                nc.scalar.activation(
                    p_sb[:, bk, :],
                    sp[:],
                    mybir.ActivationFunctionType.Exp,
                    scale=exp_scale,
                )
                if bk in bmask:
                    nc.vector.tensor_mul(p_sb[:, bk, :], p_sb[:, bk, :], bmask[bk][:])
            return dict(h=h, q0=q0, kstart=kstart, nbk=nbk, order=order, p_sb=p_sb)

        def emit_av(d):
            """AV + row-sum matmuls, normalization, attnT evacuation."""
            h, q0, kstart, nbk = d["h"], d["q0"], d["kstart"], d["nbk"]
            p_sb = d["p_sb"]
            par = (q0 // 256) % 2
            avr = ps.tile([128, 512], F32, tag=f"PS{6+par}", name="avr")
            av = avr[:, 0:256]
            rs = avr[0:1, 256:512]
            # av and rs share one psum bank as a single accumulation group.
            # start is on the first av matmul and stop on the LAST av matmul
            # (both span all 128 partitions — the group started/stopped flags
            # are per partition x zero-region, so a [1,256] rs output could
            # not start/stop the full bank). The last av is emitted after the
            # last rs so the stop is the final touch of the bank.
            n = 0
            last_av = None
            for bk in d["order"]:
                for half in range(2):
                    kt = bk * 2 + half
                    pm = r32(p_sb[:, bk, bass.ts(half, 256)])
                    last = n == 2 * nbk - 1
                    if not last:
                        nc.tensor.matmul(
                            av,
                            v_sb[:, kstart // 128 + kt, bass.ts(h, 128)],
                            pm,
                            start=(n == 0),
                            stop=False,
                        )
                    else:
                        last_av = (kt, pm)
                    nc.tensor.matmul(
                        rs[:, 0:256], r32(ones[:]), pm,
                        start=False, stop=False,
                    )
                    n += 1
            kt, pm = last_av
            nc.tensor.matmul(
                av, v_sb[:, kstart // 128 + kt, bass.ts(h, 128)], pm,
                start=False, stop=True,
            )
            rinv = rp.tile([1, 256], F32, tag=f"rinv{par}")
            nc.vector.reciprocal(rinv[:], rs[:, 0:256])
            rb = rp.tile([128, 256], F32, tag=f"rb{par}")
            nc.gpsimd.partition_broadcast(rb[:], rinv[:])
            nc.vector.tensor_mul(attnT[:, h, bass.ds(q0, 256)], av, rb[:])

        # one-stage software pipeline: scores(i+1) issued before av(i)
        prev = None
        for h in range(HC):
            for qb in range(QB):
                d = emit_scores(h, qb)
                if prev is not None:
                    emit_av(prev)
                prev = d
        emit_av(prev)

        # ---- C) out-projection (partial over this core's E dims) ----
        # evac copies alternate DVE/ACT; 1 MB output DMA per 128-row stripe
        # on the SWDGE (gpsimd) queue to keep SP free for x loads
        for st in range(ST):
            osb = outp.tile([128, D], F32, tag="osb")
            for oc in range(OC):
                o_ps = ps.tile(
                    [128, OCW], F32, tag=("PS0" if oc % 2 == 0 else "PS1"),
                    name="o_ps",
                )
                for et in range(HC):
                    nc.tensor.matmul(
                        o_ps[:],
                        r32(attnT[:, et, bass.ts(st, 128)]),
                        wo_sb[:, et, bass.ts(oc, OCW)],
                        start=(et == 0),
                        stop=(et == HC - 1),
                    )
                dst = osb[:, bass.ts(oc, OCW)]
                if oc % 2 == 0:
                    nc.vector.tensor_copy(dst, o_ps[:])
                else:
                    nc.scalar.copy(dst, o_ps[:])
            nc.gpsimd.dma_start(io["part"][b, bass.ts(st, 128), :], osb[:])


# ======================================================================
# 8-core SPMD wrapper
# ======================================================================
from contextlib import ExitStack as _ExitStack

N_CORES = 8
B_FULL, S_FULL, D_FULL, H_FULL, HD_FULL = 2, 2048, 2048, 16, 128
HC_FULL = H_FULL // N_CORES  # 2 heads per core

_nc_cache = {}


def get_compiled(reps=1):
    """Build + bacc-compile the per-core Bass program (cached per reps)."""
    if reps not in _nc_cache:
        import concourse.bacc as bacc
        from concourse import tile

        nc = bacc.Bacc(
            "TRN2", target_bir_lowering=False, debug=False, num_devices=N_CORES
        )
        io = declare_io(nc, B_FULL, S_FULL, D_FULL, HC_FULL * 128)
        with tile.TileContext(nc) as tc:
            with _ExitStack() as ctx:
                build_program(
                    ctx, nc, tc, io, B_FULL, S_FULL, D_FULL, HC_FULL, reps=reps
                )
        nc.compile()
        _nc_cache[reps] = nc
    return _nc_cache[reps]


def make_in_maps(x, w_qkv, w_out):
    """Host-side sharding: per-core input dicts (head-sharded)."""
    x = np.ascontiguousarray(np.asarray(x, dtype=np.float32))
    w_qkv = np.ascontiguousarray(np.asarray(w_qkv, dtype=np.float32))
    w_out = np.ascontiguousarray(np.asarray(w_out, dtype=np.float32))
    D = D_FULL
    xt = np.ascontiguousarray(x.transpose(0, 2, 1))
    cos_t, sin_t = host_tables(S_FULL)
    maskS, maskE = host_masks()
    in_maps = []
    for c in range(N_CORES):
        e0, e1 = c * HC_FULL * 128, (c + 1) * HC_FULL * 128
        in_maps.append(
            dict(
                xt=xt,
                wqt=np.ascontiguousarray(w_qkv[e0:e1].T),
                wkt=np.ascontiguousarray(w_qkv[D + e0 : D + e1].T),
                wvt=np.ascontiguousarray(w_qkv[2 * D + e0 : 2 * D + e1].T),
                wot=np.ascontiguousarray(w_out[:, e0:e1].T),
                cost=cos_t,
                sint=sin_t,
                maskS=maskS,
                maskE=maskE,
            )
        )
    return in_maps


def combine(parts):
    """Sum the 8 per-core out-projection partials."""
    acc = np.zeros((B_FULL, S_FULL, D_FULL), np.float64)
    for p in parts:
        acc += p
    return acc.astype(np.float32)


def kernel(x, w_qkv, w_out):
    from concourse import bass_utils

    nc = get_compiled(reps=1)
    in_maps = make_in_maps(x, w_qkv, w_out)
    res = bass_utils.run_bass_kernel_spmd(
        nc, in_maps, core_ids=list(range(N_CORES))
    )
    return combine([res.results[c]["part"] for c in range(N_CORES)])
